# revision 7
# baseline (speedup 1.0000x reference)
"""MoE post-processing MLP kernel for Trainium2 (8 NeuronCores).

Strategy: expert-parallel sharding. Each core is assigned one chunk of
samples routed to a single expert (K=8 experts ~= 8 cores for uniform
routing). The host gathers/permutes samples by expert, precomputes the
phase-reduced posenc angles (so no range reduction is needed on-chip),
and the device runs a dense 3-layer MLP entirely in fp16 SBUF tiles
(fp32 PSUM accumulation). The host scatters results back.

Device kernel (per core, C=8704 samples: 8 tiles of 1024 pair-packed
samples + one 256-col tail tile of 512):
  - ang tile [72, w]: host-side x~ = mod(angle/2pi, 1) - 0.5 in fp16
  - s36 = Sin(2pi * x~) on ScalarE (LUT domain [-pi, pi]); equals
    -sin(angle), so the W0s block is negated on the host
  - h0 = relu(W0a^T@fpv + W0s^T@s36 + b0)  (PE fp16, PSUM fp32)
  - h1 = relu(W1^T@h0 + b1);  y = W2^T@h1 + b2
  - relu0 on ScalarE, relu1 + y-copy on Vector, y DMA-out on GpSimd
All SBUF data, weights and DMA traffic are fp16 (PE runs 1 col/cycle).
Input DMAs stream on SP ahead of compute; outputs leave via the Pool
software-DGE queue so SP/ACT stay free.
"""

import numpy as np

K = 8
WID = 64
D = 32
NT = 512            # full-tile matmul moving dim (one fp32 PSUM bank)
NFULL = 8           # full tiles (1024 samples each, pair-packed)
TNT = 256           # tail-tile moving dim (512 samples)
C = NFULL * 2 * NT + 2 * TNT     # 8704 samples per core-chunk
COLS = NFULL * NT + TNT          # 4352 device columns
WIDTHS = [NT] * NFULL + [TNT]
GROUPS = [(0,), (1, 2), (3, 4), (5, 6), (7, 8)]
TWO_PI = float(2.0 * np.pi)

# W0 row indices (DIN=74 layout: feat 0:32, posenc(pos,2) 32:47,
# posenc(view,4) 47:74) for the identity part and the sin part.
_W0A_ROWS = list(range(32)) + [32, 33, 34] + [47, 48, 49]
_W0S_ROWS = (list(range(35, 41)) + list(range(50, 62))
             + list(range(41, 47)) + list(range(62, 74)))

_PREP = None  # compiled Bass program, built once per process
_LAST_IN_MAPS = None  # stashed for external profiling harnesses


def _build_program():
    import concourse.bacc as bacc
    import concourse.mybir as mybir
    from concourse.tile import TileContext

    F32, F16 = mybir.dt.float32, mybir.dt.float16
    AF = mybir.ActivationFunctionType
    ALU = mybir.AluOpType

    nc = bacc.Bacc("TRN2", target_bir_lowering=False, debug=False,
                   num_devices=8)

    fpv_d = nc.dram_tensor("fpv", [76, COLS], F16, kind="ExternalInput").ap()
    ang_d = nc.dram_tensor("ang", [72, COLS], F16, kind="ExternalInput").ap()
    wall_d = nc.dram_tensor("wall", [128, 448], F16,
                            kind="ExternalInput").ap()
    bias_d = nc.dram_tensor("bias", [128, 3], F32, kind="ExternalInput").ap()
    y_d = nc.dram_tensor("y", [64, COLS], F16, kind="ExternalOutput").ap()

    # processing order: tail group first (tiny: fast pipeline fill), then
    # the four 1024-wide pair groups.  Column ranges in the dram tensors
    # stay in tile order (full tiles at 0:4096, tail at 4096:4352).
    PGROUPS = [(4096, 256), (0, 1024), (1024, 1024),
               (2048, 1024), (3072, 1024)]

    with TileContext(nc) as tc:
        with (tc.tile_pool(name="w", bufs=1) as wp,
              tc.tile_pool(name="fp", bufs=1) as fpool,
              tc.tile_pool(name="io", bufs=3) as io,
              tc.tile_pool(name="ps0", bufs=2, space="PSUM") as ps0,
              tc.tile_pool(name="ps1", bufs=1, space="PSUM") as ps1,
              tc.tile_pool(name="psy", bufs=1, space="PSUM") as psy):
            wall = wp.tile([128, 448], F16)
            biasw = wp.tile([128, 3], F32)
            dummy = wp.tile([128, 256], F16)
            # inputs land in four big tiles (2048-wide: 4 KB DRAM rows ->
            # large DMA packets) + two small tail tiles
            at_A = fpool.tile([72, 2048], F16)
            at_B = fpool.tile([72, 2048], F16)
            at_T = fpool.tile([72, 256], F16)
            ft_A = fpool.tile([76, 2048], F16)
            ft_B = fpool.tile([76, 2048], F16)
            ft_T = fpool.tile([76, 256], F16)

            def ang_src(c0, fw):
                if c0 >= 4096:
                    return at_T[:, 0:fw]
                t = at_A if c0 < 2048 else at_B
                return t[:, c0 % 2048:c0 % 2048 + fw]

            def fpv_src(c0, fw):
                if c0 >= 4096:
                    return ft_T[:, 0:fw]
                t = ft_A if c0 < 2048 else ft_B
                return t[:, c0 % 2048:c0 % 2048 + fw]

            # DMA issue order.  SP (hardware ring): tail inputs + weights
            # first (smallest -> earliest fill), then half the bulk.
            # GpSimd (software ring, round-robin over all 16 DMA engines):
            # the other half of the bulk, then per-group outputs.
            nc.gpsimd.memset(dummy[:], 0.0)
            nc.gpsimd.dma_start(out=at_A[:], in_=ang_d[:, 0:2048])
            nc.gpsimd.dma_start(out=ft_B[:], in_=fpv_d[:, 2048:4096])
            nc.sync.dma_start(out=at_T[:], in_=ang_d[:, 4096:4352])
            nc.sync.dma_start(out=ft_T[:], in_=fpv_d[:, 4096:4352])
            nc.sync.dma_start(out=wall[:], in_=wall_d[:, :])
            nc.sync.dma_start(out=biasw[:], in_=bias_d[:, :])
            nc.sync.dma_start(out=ft_A[:], in_=fpv_d[:, 0:2048])
            nc.sync.dma_start(out=at_B[:], in_=ang_d[:, 2048:4096])

            W0at = wall[0:76, 0:128]
            W0st = wall[0:72, 128:256]
            W1t = wall[0:128, 256:384]
            W2t = wall[0:128, 384:448]
            b0t = biasw[0:128, 0:1]
            b1t = biasw[0:128, 1:2]
            b2t = biasw[0:64, 2:3]

            for gi, (gc0, fw) in enumerate(PGROUPS):
                s36 = io.tile([72, fw], F16, name="s36")
                h0t = io.tile([128, fw], F16, name="h0t")
                h1t = io.tile([128, fw], F16, name="h1t")
                yt = io.tile([64, fw], F16, name="yt")
                h0p = ps0.tile([128, fw], F32, name="h0p")
                h1p = ps1.tile([128, fw], F32, name="h1p")
                yp = psy.tile([64, fw], F32, name="yp")

                if gi == 0:
                    # p-state warmup: PE continuously busy from program start
                    # so its clock ramps to max before the real matmuls.
                    # Writes are start=True garbage into h0p, overwritten by
                    # the real accumulation below (same-engine WAW order).
                    for _ in range(12):
                        nc.tensor.matmul(out=h0p[:, 0:256],
                                         lhsT=dummy[:, 0:128], rhs=dummy[:],
                                         start=True, stop=True)

                nc.scalar.activation(s36[:], ang_src(gc0, fw), AF.Sin,
                                     bias=0.0, scale=TWO_PI)
                for off in range(0, fw, NT):
                    w = min(NT, fw - off)
                    js = slice(off, off + w)
                    nc.tensor.matmul(out=h0p[:, js], lhsT=W0at,
                                     rhs=fpv_src(gc0 + off, w),
                                     start=True, stop=False)
                    nc.tensor.matmul(out=h0p[:, js], lhsT=W0st,
                                     rhs=s36[:, js], start=False, stop=True)
                nc.scalar.activation(h0t[:], h0p[:], AF.Relu,
                                     bias=b0t, scale=1.0)
                for off in range(0, fw, NT):
                    w = min(NT, fw - off)
                    js = slice(off, off + w)
                    nc.tensor.matmul(out=h1p[:, js], lhsT=W1t,
                                     rhs=h0t[:, js], start=True, stop=True)
                nc.vector.tensor_scalar(out=h1t[:], in0=h1p[:],
                                        scalar1=b1t, scalar2=0.0,
                                        op0=ALU.add, op1=ALU.max)
                for off in range(0, fw, NT):
                    w = min(NT, fw - off)
                    js = slice(off, off + w)
                    nc.tensor.matmul(out=yp[:, js], lhsT=W2t,
                                     rhs=h1t[:, js], start=True, stop=True)
                nc.vector.tensor_scalar(out=yt[:], in0=yp[:],
                                        scalar1=b2t, scalar2=None,
                                        op0=ALU.add)
                nc.gpsimd.dma_start(out=y_d[:, gc0:gc0 + fw], in_=yt[:])

    nc.compile()
    return nc


def _get_program():
    global _PREP
    if _PREP is None:
        _PREP = _build_program()
    return _PREP


def _pack_weights(W0, b0, W1, b1, W2, b2):
    """Per-expert [128, 448] fp16 weight wall + [128, 3] f32 biases."""
    W0a = W0[_W0A_ROWS].astype(np.float32)          # [38, 64]
    W0s = -W0[_W0S_ROWS].astype(np.float32)         # [36, 64], negated
    wall = np.zeros((128, 448), np.float16)
    wall[0:38, 0:64] = W0a
    wall[38:76, 64:128] = W0a
    wall[0:36, 128:192] = W0s
    wall[36:72, 192:256] = W0s
    wall[0:64, 256:320] = W1
    wall[64:128, 320:384] = W1
    wall[0:64, 384:416] = W2
    wall[64:128, 416:448] = W2
    bias = np.zeros((128, 3), np.float32)
    bias[:, 0] = np.concatenate([b0, b0])
    bias[:, 1] = np.concatenate([b1, b1])
    bias[0:64, 2] = np.concatenate([b2, b2])
    return wall, bias


def _pack_cols(data, n):
    """[R, C-samples] -> [2R, COLS] pair-packed device layout."""
    R = data.shape[0]
    full = data[:, :NFULL * 2 * NT].reshape(R, NFULL, 2, NT)
    fullp = np.concatenate([full[:, :, 0], full[:, :, 1]],
                           axis=0).reshape(2 * R, NFULL * NT)
    tail = data[:, NFULL * 2 * NT:].reshape(R, 1, 2, TNT)
    tailp = np.concatenate([tail[:, :, 0], tail[:, :, 1]],
                           axis=0).reshape(2 * R, TNT)
    return np.concatenate([fullp, tailp], axis=1)


def _unpack_cols(y):
    """[64, COLS] device layout -> [32, C] sample order."""
    yf = y[:, :NFULL * NT].reshape(64, NFULL, NT)
    full = np.stack([yf[0:32], yf[32:64]], axis=2).reshape(32, NFULL * 2 * NT)
    ytl = y[:, NFULL * NT:].reshape(64, 1, TNT)
    tail = np.stack([ytl[0:32], ytl[32:64]], axis=2).reshape(32, 2 * TNT)
    return np.concatenate([full, tail], axis=1)


def kernel(idxs, positions, viewdirs, features, W0, b0, W1, b1, W2, b2):
    from concourse.bass_utils import run_bass_kernel_spmd

    N = idxs.shape[0]
    idx = idxs.reshape(-1).astype(np.int64)
    out = np.zeros((N, D), np.float32)

    # Route: list of (expert, sample-index-array) chunks of <= C samples.
    chunks = []
    for k in range(K):
        sel = np.nonzero(idx == k)[0]
        for lo in range(0, len(sel), C):
            chunks.append((k, sel[lo:lo + C]))

    walls = [_pack_weights(W0[k], b0[k], W1[k], b1[k], W2[k], b2[k])
             for k in range(K)]

    inv_2pi = np.float32(1.0 / TWO_PI)
    scales = np.array([1, 2, 1, 2, 4, 8], np.float32) * inv_2pi

    nc = _get_program()
    zero_in = None
    for inv in range(0, len(chunks), 8):
        batch = chunks[inv:inv + 8]
        in_maps = []
        for ci in range(8):
            if ci < len(batch):
                k, sel = batch[ci]
                n = len(sel)
                fpv38 = np.zeros((38, C), np.float16)
                fpv38[0:32, :n] = features[sel].T
                fpv38[32:35, :n] = positions[sel].T
                fpv38[35:38, :n] = viewdirs[sel].T
                # phase-reduced angles: x~ = mod(m*x/2pi + phase, 1) - 0.5
                pv = np.concatenate([positions[sel], positions[sel],
                                     viewdirs[sel], viewdirs[sel],
                                     viewdirs[sel], viewdirs[sel]],
                                    axis=1).astype(np.float32)  # [n, 18]
                u18 = pv * np.repeat(scales, 3)[None, :]
                x36 = np.concatenate([u18, u18 + np.float32(0.25)], axis=1)
                x36 = (x36 - np.floor(x36)) - np.float32(0.5)
                ang36 = np.zeros((36, C), np.float16)
                ang36[:, :n] = x36.T
                in_maps.append({"fpv": np.ascontiguousarray(
                                    _pack_cols(fpv38, n)),
                                "ang": np.ascontiguousarray(
                                    _pack_cols(ang36, n)),
                                "wall": walls[k][0],
                                "bias": walls[k][1]})
            else:
                if zero_in is None:
                    zero_in = {"fpv": np.zeros((76, COLS), np.float16),
                               "ang": np.zeros((72, COLS), np.float16),
                               "wall": walls[0][0],
                               "bias": walls[0][1]}
                in_maps.append(zero_in)
        global _LAST_IN_MAPS
        _LAST_IN_MAPS = in_maps
        res = None
        for attempt in range(3):
            try:
                res = run_bass_kernel_spmd(nc, in_maps,
                                           core_ids=list(range(8)))
                break
            except Exception:
                if attempt == 2:
                    raise
        assert res is not None
        for ci, (k, sel) in enumerate(batch):
            y64 = np.asarray(res.results[ci]["y"], np.float32)  # [64, COLS]
            y32 = _unpack_cols(y64)
            out[sel] = y32[:, :len(sel)].T
    return out
